# revision 2
# baseline (speedup 1.0000x reference)
"""Multi-head graph attention (GAT) Trainium2 kernel.

Row-sharded across 8 NeuronCores: core i owns queries [i*1024, (i+1)*1024).

Math (per head h, with Wh = h @ W_h, a = Wh@a1, b = Wh@a2):
    e[i,j]  = leakyrelu(a_i + b_j, 0.2)
    attn    = softmax_j(where(adj>0, e, -9e15))
    out_h   = elu(attn @ Wh)
    out     = concat_h(out_h) @ Wp.T + bp

Exact on-chip factorization (ea02_i cancels in softmax normalization):
    w[i,j] = adj[i,j] * max(exp(0.8 a_i) * exp(b_j), exp(0.2 b_j))
so per (key-block, head) the masked weights need one tensor_scalar
(P = ea08 * eb) and one scalar_tensor_tensor ((P max v2) * mask).
The mask arrives pre-transposed as bf16 from the host (keys on
partitions), so there is no DMA transpose and no on-chip cast.

elu is computed as elu(x)+1 = max(x,0) + exp(min(x,0)); the -1 is
folded into the output bias (bp' = bp - Wp.sum(1)) on the host.
"""

import os
from contextlib import ExitStack

import numpy as np

import concourse.bacc as bacc
import concourse.bass as bass
import concourse.mybir as mybir
import concourse.tile as tile

F32 = mybir.dt.float32
BF16 = mybir.dt.bfloat16

ALU = mybir.AluOpType
AF = mybir.ActivationFunctionType

N = 8192          # nodes
IN_F = 256        # input features
H = 4             # heads
DH = 64           # head dim
NCORES = 8
QN = N // NCORES  # queries per core (1024)
KB = N // 128     # key blocks of 128 (64)
QH = QN // 512    # 512-wide query halves per core (2)
MG = 4            # mask DMA granularity (key blocks per DMA)


def build_nc():
    nc = bacc.Bacc("TRN2", target_bir_lowering=False, debug=False)

    ht = nc.declare_dram_parameter("ht", [IN_F, N], F32, False)       # h.T (replicated)
    hqt = nc.declare_dram_parameter("hqt", [IN_F, QN], F32, False)    # h.T query slice
    adjt = nc.declare_dram_parameter("adjt", [N, QN], BF16, False)    # adj[qsl,:].T as bf16 0/1
    wam = nc.declare_dram_parameter("wam", [IN_F, IN_F + 8], F32, False)  # [W_all | a1~ | a2~]
    wpt = nc.declare_dram_parameter("wpt", [IN_F, IN_F], F32, False)  # Wp.T
    bpp = nc.declare_dram_parameter("bpp", [IN_F], F32, False)        # bp - Wp.sum(1)
    out = nc.declare_dram_parameter("out", [QN, IN_F], F32, True)

    # per-head g-op placement: 'dve' (ts P=ea*eb; stt (P max v2)*m),
    # 'act' (ACT relu; stt (g+v2)*m), 'gps' (ts on gpsimd; stt),
    # 'dve2' (dual-op ts g=max(ea*eb,v2); TT pm=g*m)
    FORMS = os.environ.get("GAT_FORMS", "dve,act,dve,act").split(",")
    assert len(FORMS) == H

    with ExitStack() as ctx:
        tc = ctx.enter_context(tile.TileContext(nc))

        persist = ctx.enter_context(tc.tile_pool(name="persist", bufs=1))
        # stationaries: [k-part, kblock, head, dh+1] holding raw [Wh | 1]
        whv = persist.tile([128, KB, H, DH + 1], BF16)
        # per-key factors (per-partition scalars): eb = exp(b), v2 = exp(0.2 b)
        eb = persist.tile([128, H, KB], F32)
        v2 = persist.tile([128, H, KB], F32)
        nv2 = persist.tile([128, H, KB], F32)
        braw = persist.tile([128, H, KB], F32)
        # per-query exp(0.8 a) broadcast across partitions
        ea08b = persist.tile([128, H, QN], BF16)
        wpt_sb = persist.tile([128, 2, IN_F], F32)
        bpb = persist.tile([128, IN_F], F32)
        ones1 = persist.tile([1, 128], BF16)
        ones_f = persist.tile([1, 64], F32)

        # main-loop pools pinned before setup so their SBUF slots never
        # alias setup tiles (avoids false WAR deps gating the pipeline).
        mloop = ctx.enter_context(tc.tile_pool(name="mloop", bufs=3))
        for _b in range(3):
            _t = mloop.tile([128, MG, QN], BF16, tag="mask")
            nc.vector.memset(_t[0:1, 0, 0:2], 0.0)
        gpool = ctx.enter_context(tc.tile_pool(name="gpool", bufs=6))
        for _b in range(6):
            _t = gpool.tile([128, QN], BF16, tag="g")
            nc.vector.memset(_t[0:1, 0:2], 0.0)
            _t = gpool.tile([128, QN], BF16, tag="pm")
            nc.vector.memset(_t[0:1, 0:2], 0.0)

        # ---------------- setup phase ----------------
        with tc.tile_pool(name="setup", bufs=1) as setup, \
             tc.tile_pool(name="htp", bufs=2) as htp, \
             tc.tile_pool(name="spsum", bufs=4, space="PSUM") as spsum, \
             tc.tile_pool(name="spsum2", bufs=2, space="PSUM") as spsum2:
            nc.vector.memset(ones1, 1.0)
            nc.vector.memset(ones_f, 1.0)
            nc.vector.memset(whv[:, :, :, DH:DH + 1], 1.0)

            wam_sb = setup.tile([128, 2, IN_F + 8], F32)
            nc.scalar.dma_start(wam_sb, wam[:, :].rearrange("(c p) w -> p c w", p=128))
            nc.scalar.dma_start(wpt_sb, wpt[:, :].rearrange("(c p) w -> p c w", p=128))
            bp_ap = bpp[:]
            nc.gpsimd.dma_start(bpb, bass.AP(tensor=bp_ap.tensor, offset=bp_ap.offset,
                                             ap=[[0, 128]] + list(bp_ap.ap)))

            hqt_sb = setup.tile([128, 2, QN], F32)
            nc.scalar.dma_start(hqt_sb, hqt[:, :].rearrange("(c p) n -> p c n", p=128))

            # a-scores: exp(0.8 a) rows -> broadcast tiles (main loop needs
            # these first, so they are emitted first).
            ea08r = setup.tile([1, H, QN], BF16)
            for h in range(H):
                for qh in range(QH):
                    qsl = slice(qh * 512, (qh + 1) * 512)
                    pa = spsum2.tile([1, 512], F32, tag="a_ps")
                    nc.tensor.matmul(pa, wam_sb[:, 0, IN_F + h:IN_F + h + 1],
                                     hqt_sb[:, 0, qsl], start=True, stop=False)
                    nc.tensor.matmul(pa, wam_sb[:, 1, IN_F + h:IN_F + h + 1],
                                     hqt_sb[:, 1, qsl], start=False, stop=True)
                    nc.scalar.activation(ea08r[:, h, qsl], pa, AF.Exp, scale=0.8)
                    pb2 = spsum2.tile([128, 512], F32, tag="b_ps")
                    nc.tensor.matmul(pb2, ones1, ea08r[:, h, qsl])
                    nc.vector.tensor_copy(ea08b[:, h, qsl], pb2)

            # Wh (raw, bf16) + raw b-scores per key chunk; exp factors per
            # ht quarter so the main loop can start early.
            ht_r = ht[:, :].rearrange("(c p) n -> p c n", p=128)
            for i in range(4):
                htq = htp.tile([128, 2, N // 4], F32, tag="htq")
                nsl = slice(i * (N // 4), (i + 1) * (N // 4))
                nc.scalar.dma_start(htq, ht_r[:, :, nsl])
                for kq in range(16):
                    kc = i * 16 + kq
                    ps = spsum.tile([128, IN_F + 8], F32, tag="wh_ps")
                    ksl = slice(kq * 128, (kq + 1) * 128)
                    nc.tensor.matmul(ps, htq[:, 0, ksl], wam_sb[:, 0, :],
                                     start=True, stop=False)
                    nc.tensor.matmul(ps, htq[:, 1, ksl], wam_sb[:, 1, :],
                                     start=False, stop=True)
                    nc.vector.tensor_copy(braw[:, :, kc:kc + 1],
                                          ps[:, IN_F + 4:IN_F + 8].rearrange(
                                              "p (h o) -> p h o", o=1))
                    if kc % 2 == 0:
                        nc.scalar.copy(
                            whv[:, kc, :, 0:DH],
                            ps[:, 0:IN_F].rearrange("p (h d) -> p h d", h=H))
                    else:
                        nc.vector.tensor_copy(
                            whv[:, kc, :, 0:DH],
                            ps[:, 0:IN_F].rearrange("p (h d) -> p h d", h=H))
                bsl = slice(i * 16, (i + 1) * 16)
                nc.scalar.activation(eb[:, :, bsl], braw[:, :, bsl], AF.Exp)
                nc.scalar.activation(v2[:, :, bsl], braw[:, :, bsl], AF.Exp, scale=0.2)
                nc.vector.tensor_scalar(nv2[:, :, bsl], v2[:, :, bsl], -1.0, None,
                                        op0=ALU.mult)

        # ---------------- main loop ----------------
        mpsum_cm = tc.tile_pool(name="mpsum", bufs=1, space="PSUM")
        mpsum = mpsum_cm.__enter__()
        acc = mpsum.tile([DH + 1, H, QH, 512], F32)

        for kb4 in range(KB // MG):
            mask4 = mloop.tile([128, MG, QN], BF16, tag="mask")
            nc.sync.dma_start(
                mask4,
                adjt[kb4 * MG * 128:(kb4 + 1) * MG * 128, :].rearrange(
                    "(j p) q -> p j q", p=128))
            for j in range(MG):
                kb = kb4 * MG + j
                mt = mask4[:, j, :]
                for h in range(H):
                    form = FORMS[h]
                    pm = gpool.tile([128, QN], BF16, tag="pm")
                    if form == "act":
                        g = gpool.tile([128, QN], BF16, tag="g")
                        nc.scalar.activation(g, ea08b[:, h, :], AF.Relu,
                                             bias=nv2[:, h, kb:kb + 1],
                                             scale=eb[:, h, kb:kb + 1])
                        nc.vector.scalar_tensor_tensor(
                            pm, g, v2[:, h, kb:kb + 1], mt,
                            op0=ALU.add, op1=ALU.mult)
                    elif form == "dve2":
                        g = gpool.tile([128, QN], BF16, tag="g")
                        nc.vector.tensor_scalar(
                            g, ea08b[:, h, :], eb[:, h, kb:kb + 1],
                            v2[:, h, kb:kb + 1], op0=ALU.mult, op1=ALU.max)
                        nc.vector.tensor_mul(pm, g, mt)
                    else:
                        g = gpool.tile([128, QN], BF16, tag="g")
                        eng = nc.gpsimd if form == "gps" else nc.vector
                        eng.tensor_scalar(g, ea08b[:, h, :], eb[:, h, kb:kb + 1],
                                          None, op0=ALU.mult)
                        nc.vector.scalar_tensor_tensor(
                            pm, g, v2[:, h, kb:kb + 1], mt,
                            op0=ALU.max, op1=ALU.mult)
                    for qh in range(QH):
                        nc.tensor.matmul(acc[:, h, qh, :], whv[:, kb, h, :],
                                         pm[:, qh * 512:(qh + 1) * 512],
                                         start=(kb == 0), stop=(kb == KB - 1))

        # ---------------- tail: normalize, elu, out-proj ----------------
        tailp = ctx.enter_context(tc.tile_pool(name="tailp", bufs=1))
        denln = tailp.tile([1, H, QN], F32)
        rden = tailp.tile([1, H, QN], F32)
        graw = tailp.tile([128, 2, QN], F32)
        gfin = tailp.tile([128, 2, QN], F32)

        for h in range(H):
            for qh in range(QH):
                qsl = slice(qh * 512, (qh + 1) * 512)
                nc.scalar.activation(denln[:, h, qsl], acc[DH:DH + 1, h, qh, :],
                                     AF.Ln)
            # raw (unnormalized) h'.T for head h -> partitions [(h%2)*64, ...)
            nc.vector.tensor_copy(
                graw[(h % 2) * 64:(h % 2) * 64 + 64, h // 2, :],
                acc[0:DH, h, :, :].rearrange("p a b -> p (a b)"))
        nc.scalar.activation(rden, denln, AF.Exp, scale=-1.0)
        mpsum_cm.__exit__(None, None, None)

        with tc.tile_pool(name="tpsum", bufs=2, space="PSUM") as tpsum:
            # normalize: broadcast 1/den across partitions via ones-matmul,
            # then fused elu: gfin = max(gn,0) + exp(min(gn,0))  (-1 is in bpp)
            for j in range(2):
                for qh in range(QH):
                    qsl = slice(qh * 512, (qh + 1) * 512)
                    rps = tpsum.tile([128, 512], F32, tag="r_ps")
                    nc.tensor.matmul(rps[0:64, :], ones_f, rden[:, 2 * j, qsl])
                    nc.tensor.matmul(rps[64:128, :], ones_f, rden[:, 2 * j + 1, qsl])
                    gn = tailp.tile([128, 512], F32, tag="gn")
                    nc.vector.tensor_mul(gn, graw[:, j, qsl], rps)
                    t = tailp.tile([128, 512], F32, tag="elu_t")
                    nc.vector.tensor_scalar(t, gn, 0.0, None, op0=ALU.min)
                    e = tailp.tile([128, 512], F32, tag="elu_e")
                    nc.scalar.activation(e, t, AF.Exp)
                    nc.vector.scalar_tensor_tensor(gfin[:, j, qsl], gn,
                                                   0.0, e, op0=ALU.max, op1=ALU.add)

            for qc in range(QN // 128):
                qsl = slice(qc * 128, (qc + 1) * 128)
                po = tpsum.tile([128, IN_F], F32, tag="out_ps")
                nc.tensor.matmul(po, gfin[:, 0, qsl], wpt_sb[:, 0, :],
                                 start=True, stop=False)
                nc.tensor.matmul(po, gfin[:, 1, qsl], wpt_sb[:, 1, :],
                                 start=False, stop=True)
                fin = tailp.tile([128, IN_F], F32, tag="fin")
                nc.vector.scalar_tensor_tensor(fin, po, 0.0, bpb,
                                               op0=ALU.add, op1=ALU.add)
                nc.sync.dma_start(out[qsl, :], fin)

    nc.compile()
    return nc


_NC_CACHE = {}
LAST_RESULTS = None


def _get_nc():
    if "nc" not in _NC_CACHE:
        _NC_CACHE["nc"] = build_nc()
    return _NC_CACHE["nc"]


def kernel(h, adj, W, a1, a2, Wp, bp):
    from concourse.bass_utils import run_bass_kernel_spmd

    h = np.asarray(h, dtype=np.float32)
    adj = np.asarray(adj)
    W = np.asarray(W, dtype=np.float32)
    a1 = np.asarray(a1, dtype=np.float32)
    a2 = np.asarray(a2, dtype=np.float32)
    Wp = np.asarray(Wp, dtype=np.float32)
    bp = np.asarray(bp, dtype=np.float32)

    # host-side parameter marshaling
    W_all = np.ascontiguousarray(W.transpose(1, 0, 2).reshape(IN_F, H * DH))
    amat_a = np.einsum("hid,hd->ih", W, a1)  # [256, 4]: h @ amat_a = a scores
    amat_b = np.einsum("hid,hd->ih", W, a2)  # [256, 4]
    wam = np.ascontiguousarray(
        np.concatenate([W_all, amat_a, amat_b], axis=1).astype(np.float32))
    ht = np.ascontiguousarray(h.T)
    wpt = np.ascontiguousarray(Wp.T)
    bpp = (bp - Wp.sum(axis=1)).astype(np.float32)  # elu's -1 folded in

    # adj columns-per-core, transposed, as bf16 bit patterns (1.0 = 0x3F80)
    import ml_dtypes
    adj_bits = (adj != 0).astype(np.uint16) * np.uint16(0x3F80)

    nc = _get_nc()
    in_maps = []
    for c in range(NCORES):
        qsl = slice(c * QN, (c + 1) * QN)
        in_maps.append({
            "ht": ht,
            "hqt": np.ascontiguousarray(ht[:, qsl]),
            "adjt": np.ascontiguousarray(adj_bits[qsl, :].T).view(ml_dtypes.bfloat16),
            "wam": wam,
            "wpt": wpt,
            "bpp": bpp,
        })

    res = run_bass_kernel_spmd(nc, in_maps, core_ids=list(range(NCORES)))
    global LAST_RESULTS
    LAST_RESULTS = res
    return np.concatenate([r["out"] for r in res.results], axis=0)
